# revision 6
# baseline (speedup 1.0000x reference)
"""MoE routing kernel for TRN2 (8 NeuronCores), Bass/Tile.

Strategy: data-parallel over batch (8 samples per core; every core handles
all 4 gates for its samples). Gating (avg-pool -> softmax -> top-k ->
renorm) is computed on host with eager jnp ops mirroring the reference
op-for-op, so routing decisions match bit-exactly. Per-(gate,sample,top)
expert weights are gathered on host into one packed [128,1024] f32 panel
per chain and streamed to the device; the device program is fully static.

Per chain (gate g, sample s, top t with expert e):
  mm1: h1[hd,p]  = W1[e] @ X[s]          (K=C=128,  M=HD=256, N=P=1024)
  c1 : h1 += b1[e], PSUM->SBUF
  mm2: h2[g2,p]  = W2[e] @ h1            (K=HD=256, M=HD=256, N=1024)
  c2 : h2 = relu(h2*scaleA + biasA)      scaleA = tw*gamma*rsqrt(var+eps)
                                          biasA = tw*((b2-mean)*inv+beta)
  mm3: y[c,p]   += W3[e] @ h2            accumulated in PSUM over t=0,1
  fin: out = y + (tw0*b3[e0]+tw1*b3[e1]), PSUM->SBUF, DMA out
All matmuls run in float32r (~bf16 speed, ~1.5e-4 rel err).
"""
import functools

import numpy as np

E, TOP, C, HD, B, H, W_, NG = 8, 2, 128, 256, 64, 32, 32, 4
P = H * W_            # 1024
NCORES = 8
SPC = B // NCORES     # samples per core: 8
CHAINS = SPC * NG     # (g,s) pairs per core: 32
EPS = 1e-5
NH = 512              # matmul free-dim chunk

# packed weight panel columns: W1T[0:256] | W2T_k0[256:512] | W2T_k1[512:768]
# | W3T_k0[768:896] | W3T_k1[896:1024]
WCOLS = 1024
MCOLS = 13 * CHAINS   # meta columns


@functools.lru_cache(maxsize=1)
def _build_program():
    from concourse import bacc, mybir
    import concourse.tile as tile

    f32 = mybir.dt.float32
    f32r = mybir.dt.float32r
    nc = bacc.Bacc("TRN2", target_bir_lowering=False, debug=False)

    x_d = nc.dram_tensor("x", [SPC, C, P], f32r, kind="ExternalInput")
    wp_d = nc.dram_tensor("wp", [2 * CHAINS, C, WCOLS], f32r, kind="ExternalInput")
    meta_d = nc.dram_tensor("meta", [C, MCOLS], f32, kind="ExternalInput")
    out_d = nc.dram_tensor("out", [CHAINS, C, P], f32, kind="ExternalOutput")

    with tile.TileContext(nc) as tc:
        with tc.tile_pool(name="xs", bufs=2) as xpool, \
             tc.tile_pool(name="wt", bufs=4) as wpool, \
             tc.tile_pool(name="h1", bufs=4) as h1pool, \
             tc.tile_pool(name="h2", bufs=4) as h2pool, \
             tc.tile_pool(name="osb", bufs=3) as opool, \
             tc.tile_pool(name="mt", bufs=1) as mpool, \
             tc.tile_pool(name="ps", bufs=6, space="PSUM") as pspool, \
             tc.tile_pool(name="py", bufs=2, space="PSUM") as pypool:

            meta = mpool.tile([C, MCOLS], f32)
            nc.sync.dma_start(out=meta[:], in_=meta_d[:])

            for s in range(SPC):
                xs = xpool.tile([C, P], f32r, tag="xs")
                nc.sync.dma_start(out=xs[:], in_=x_d[s])
                for g in range(NG):
                    j = s * NG + g
                    mb = 13 * j
                    psY = [pypool.tile([C, NH], f32, tag="psY", name=f"psY{n}")
                           for n in range(2)]
                    for t in range(2):
                        tb = mb + 6 * t
                        wt = wpool.tile([C, WCOLS], f32r, tag="wt")
                        nc.sync.dma_start(out=wt[:], in_=wp_d[2 * j + t])

                        # ---- mm1: h1 = W1T.T @ X ----
                        ps1 = [[None, None], [None, None]]
                        for m in range(2):
                            lhs = wt[:, m * 128:(m + 1) * 128]
                            for n in range(2):
                                pt = pspool.tile([C, NH], f32, tag="ps", name=f"ps1_{m}{n}")
                                nc.tensor.matmul(
                                    pt[:], lhs, xs[:, n * NH:(n + 1) * NH],
                                    start=True, stop=True)
                                ps1[m][n] = pt
                        # c1: += b1 chunk, to SBUF (f32r: producer rounds)
                        h1 = [h1pool.tile([C, P], f32r, tag="h1", name=f"h1_{m}")
                              for m in range(2)]
                        for m in range(2):
                            b1ap = meta[:, tb + m:tb + m + 1]
                            for n in range(2):
                                nc.vector.tensor_scalar_add(
                                    out=h1[m][:, n * NH:(n + 1) * NH],
                                    in0=ps1[m][n][:], scalar1=b1ap)

                        # ---- mm2: h2 = W2T.T @ h1 (accumulate over k) ----
                        ps2 = [[None, None], [None, None]]
                        for m in range(2):
                            for n in range(2):
                                ps2[m][n] = pspool.tile([C, NH], f32, tag="ps", name=f"ps2_{m}{n}")
                        for k in range(2):
                            for m in range(2):
                                lhs = wt[:, 256 + k * 256 + m * 128:
                                         256 + k * 256 + (m + 1) * 128]
                                for n in range(2):
                                    rhs = h1[k][:, n * NH:(n + 1) * NH]
                                    nc.tensor.matmul(
                                        ps2[m][n][:], lhs, rhs,
                                        start=(k == 0), stop=(k == 1))
                        # c2: relu(scaleA*. + biasA) -> SBUF
                        h2 = [h2pool.tile([C, P], f32r, tag="h2", name=f"h2_{m}")
                              for m in range(2)]
                        for m in range(2):
                            sA = meta[:, tb + 2 + m:tb + 3 + m]
                            bA = meta[:, tb + 4 + m:tb + 5 + m]
                            for n in range(2):
                                nc.scalar.activation(
                                    out=h2[m][:, n * NH:(n + 1) * NH],
                                    in_=ps2[m][n][:],
                                    func=mybir.ActivationFunctionType.Relu,
                                    bias=bA, scale=sA)

                        # ---- mm3: psY += W3T.T @ h2 (accum over k and t) ----
                        for k in range(2):
                            lhs = wt[:, 768 + k * 128:768 + (k + 1) * 128]
                            for n in range(2):
                                rhs = h2[k][:, n * NH:(n + 1) * NH]
                                nc.tensor.matmul(
                                    psY[n][:], lhs, rhs,
                                    start=(t == 0 and k == 0),
                                    stop=(t == 1 and k == 1))

                    # fin: out = psY + bias3 -> SBUF -> DRAM
                    b3ap = meta[:, mb + 12:mb + 13]
                    osb = opool.tile([C, P], f32, tag="osb")
                    for n in range(2):
                        nc.vector.tensor_scalar_add(
                            out=osb[:, n * NH:(n + 1) * NH],
                            in0=psY[n][:], scalar1=b3ap)
                    nc.sync.dma_start(out=out_d[j], in_=osb[:])

    nc.compile()
    return nc


def _gating(x, gates):
    """Host gating, eager jnp op-for-op as the reference (bit-exact routing).

    Returns top_i [NG,B,TOP] int, tw [NG,B,TOP] f32 (renormalized weights).
    """
    import jax
    import jax.numpy as jnp

    xj = jnp.asarray(x)
    gj = jnp.asarray(gates)
    x0 = xj.mean(axis=(2, 3))                      # [B, C]
    tis, tws = [], []
    for i in range(NG):
        probs = jax.nn.softmax(x0 @ gj[i], axis=1)  # [B, E]
        top_p, top_i = jax.lax.top_k(probs, TOP)    # [B, TOP]
        tw = jax.nn.softmax(top_p, axis=1)          # [B, TOP]
        tis.append(np.asarray(top_i))
        tws.append(np.asarray(tw).astype(np.float32))
    return np.stack(tis), np.stack(tws)


def build_in_maps(inputs):
    """Host-side prep: gating + per-core input maps. inputs: name->np array."""
    x = np.asarray(inputs["x"], dtype=np.float32)
    gates = np.asarray(inputs["gates"], dtype=np.float32)
    W1 = np.asarray(inputs["W1"], dtype=np.float32)
    b1 = np.asarray(inputs["b1"], dtype=np.float32)
    W2 = np.asarray(inputs["W2"], dtype=np.float32)
    b2 = np.asarray(inputs["b2"], dtype=np.float32)
    bn_gamma = np.asarray(inputs["bn_gamma"], dtype=np.float32)
    bn_beta = np.asarray(inputs["bn_beta"], dtype=np.float32)
    bn_mean = np.asarray(inputs["bn_mean"], dtype=np.float32)
    bn_var = np.asarray(inputs["bn_var"], dtype=np.float32)
    W3 = np.asarray(inputs["W3"], dtype=np.float32)
    b3 = np.asarray(inputs["b3"], dtype=np.float32)
    top_i, tw = _gating(x, gates)  # [NG,B,TOP]

    # packed per-expert weight panels [E, C, WCOLS]
    packs = np.empty((E, C, WCOLS), dtype=np.float32)
    for e in range(E):
        w1t = W1[e].T                    # [C, HD]
        w2t = W2[e].T                    # [HD, HD] rows=h_in
        w3t = W3[e].T                    # [HD, C]
        packs[e, :, 0:256] = w1t
        packs[e, :, 256:512] = w2t[0:128, :]
        packs[e, :, 512:768] = w2t[128:256, :]
        packs[e, :, 768:896] = w3t[0:128, :]
        packs[e, :, 896:1024] = w3t[128:256, :]

    inv = bn_gamma / np.sqrt(bn_var + np.float32(EPS))   # [E, HD]
    biasA_e = (b2 - bn_mean) * inv + bn_beta             # [E, HD]

    xr = x.reshape(B, C, P)
    in_maps = []
    for c in range(NCORES):
        s0 = c * SPC
        eid = np.empty(2 * CHAINS, dtype=np.int64)
        meta = np.zeros((C, MCOLS), dtype=np.float32)
        for s in range(SPC):
            for g in range(NG):
                j = s * NG + g
                mb = 13 * j
                bias3 = np.zeros(C, dtype=np.float32)
                for t in range(2):
                    e = int(top_i[g, s0 + s, t])
                    w = tw[g, s0 + s, t]
                    eid[2 * j + t] = e
                    tb = mb + 6 * t
                    meta[:, tb + 0] = b1[e, 0:128]
                    meta[:, tb + 1] = b1[e, 128:256]
                    meta[:, tb + 2] = inv[e, 0:128] * w
                    meta[:, tb + 3] = inv[e, 128:256] * w
                    meta[:, tb + 4] = biasA_e[e, 0:128] * w
                    meta[:, tb + 5] = biasA_e[e, 128:256] * w
                    bias3 += w * b3[e]
                meta[:, mb + 12] = bias3
        in_maps.append({
            "x": np.ascontiguousarray(xr[s0:s0 + SPC]),
            "wp": packs[eid],
            "meta": meta,
        })
    return in_maps


def kernel(x, gates, W1, b1, W2, b2, bn_gamma, bn_beta, bn_mean, bn_var,
           W3, b3):
    from concourse.bass_utils import run_bass_kernel_spmd

    in_maps = build_in_maps({
        "x": x, "gates": gates, "W1": W1, "b1": b1, "W2": W2, "b2": b2,
        "bn_gamma": bn_gamma, "bn_beta": bn_beta, "bn_mean": bn_mean,
        "bn_var": bn_var, "W3": W3, "b3": b3,
    })
    nc = _build_program()
    res = run_bass_kernel_spmd(nc, in_maps, list(range(NCORES)))

    outs = []
    for g in range(NG):
        og = np.empty((B, C, P), dtype=np.float32)
        for b in range(B):
            c, s = divmod(b, SPC)
            og[b] = res.results[c]["out"][s * NG + g]
        outs.append(og.reshape(B, C, H, W_))
    return tuple(outs)
